# revision 13
# baseline (speedup 1.0000x reference)
"""Causal self-attention (B=4, T=2048, C=1024, 16 heads) on 8 trn2 NeuronCores.

Sharding: core c = (batch c//2, head-group c%2 of 8 heads). Data-parallel over
batch, tensor-parallel over heads; out-proj is row-sharded and the two partial
products per batch are summed on the host (no device collectives).

Device program per core (bf16 GEMM inputs, fp32 PSUM):
  QKV chunks (t-chunks of 512) are WOVEN into the attention loop so the
  scalar-engine exp stream overlaps phase-1 PE work:
    qkv(0); for qc: interleave(attn(qc)+flush+proj(qc-1), qkv(qc+1)); proj(3)
  - q^T/k^T head-pair tiles [128, T] (d on partitions, both heads packed,
    no zero padding; S matmuls contract K=64 via base-partition slices)
  - V in natural [t, d] layout with a ones column per head (bias trick) so
    P@V also accumulates softmax row-sums for free
  - S computed transposed (no PE transposes, no max subtraction: |S| < ~3),
    causally trimmed matmuls + single static 128x128 triangular mask
  - normalization: vector reciprocal straight off PSUM + gpsimd partition
    broadcast, folded into the PSUM->yT multiply (no DMA round trips)
  - out-proj tiles are emitted per chunk, woven into the next chunk's
    attention, output DMA spread across the kernel
"""

import os
import sys

import numpy as np

for _p in ("/opt/trn_rl_repo", "/root/.axon_site/_ro/trn_rl_repo"):
    if os.path.isdir(_p) and _p not in sys.path:
        sys.path.insert(0, _p)

import concourse.bass as bass  # noqa: E402
import concourse.tile as tile  # noqa: E402
from concourse import bacc, mybir  # noqa: E402
from concourse.bass_utils import run_bass_kernel_spmd  # noqa: E402

B, T, C = 4, 2048, 1024
H, D = 16, 64
N_CORES = 8
F32 = mybir.dt.float32
BF16 = mybir.dt.bfloat16
TC = T // 512  # 4 t-chunks of 512
TT = T // 128  # 16 t-tiles of 128
CT = C // 128  # 8 c-tiles of 128

_cache: dict = {}


def _emit(nc: "bacc.Bacc", tc: "tile.TileContext", d: dict) -> None:
    mult = mybir.AluOpType.mult
    add = mybir.AluOpType.add
    Exp = mybir.ActivationFunctionType.Exp
    dma = nc.sync.dma_start

    cpool = tc.alloc_tile_pool(name="const", bufs=1)
    persist = tc.alloc_tile_pool(name="persist", bufs=1)
    wpool = tc.alloc_tile_pool(name="wts", bufs=1)
    xpool = tc.alloc_tile_pool(name="xt", bufs=2)
    espool = tc.alloc_tile_pool(name="es", bufs=8)
    rpool = tc.alloc_tile_pool(name="rc", bufs=2)
    opool = tc.alloc_tile_pool(name="ob", bufs=3)
    pss = tc.alloc_tile_pool(name="pss", bufs=2, space="PSUM")
    psy = tc.alloc_tile_pool(name="psy", bufs=1, space="PSUM")
    pgen = tc.alloc_tile_pool(name="pgen", bufs=2, space="PSUM")

    # ---- weights + small consts (first DMAs feed the first q/k groups) ----
    wq_sb = wpool.tile([128, CT, 512], BF16, name="wq", tag="wq")
    wk_sb = wpool.tile([128, CT, 512], BF16, name="wk", tag="wk")
    wv_sb = wpool.tile([128, CT, 520], BF16, name="wv", tag="wv")
    wp_sb = wpool.tile([128, 4, 1024], BF16, name="wp", tag="wp")
    xts = [xpool.tile([128, CT, 512], BF16, name="xt", tag="xt") for _ in range(2)]

    dma(out=wq_sb[:, :, 0:128], in_=d["wq"].ap()[:, 0:128].rearrange("(ct p) n -> p ct n", p=128))
    dma(out=xts[0][:, 0:4, :], in_=d["xT"].ap()[0:512, 0:512].rearrange("(ct p) n -> p ct n", p=128))
    dma(out=xts[0][:, 4:8, :], in_=d["xT"].ap()[512:1024, 0:512].rearrange("(ct p) n -> p ct n", p=128))
    dma(out=wq_sb[:, :, 128:512], in_=d["wq"].ap()[:, 128:512].rearrange("(ct p) n -> p ct n", p=128))
    dma(out=wk_sb[:], in_=d["wk"].ap().rearrange("(ct p) n -> p ct n", p=128))
    dma(out=wv_sb[:], in_=d["wv"].ap().rearrange("(ct p) n -> p ct n", p=128))

    m01_sb = cpool.tile([128, 128], BF16, name="m01", tag="m01")
    dma(out=m01_sb[:], in_=d["m01"].ap())
    warm_sb = cpool.tile([128, 2], F32, name="warm", tag="warm")
    nc.vector.memset(warm_sb[:, 0:1], 0.0)
    nc.scalar.activation(warm_sb[:, 1:2], warm_sb[:, 0:1], Exp)
    bq_sb = cpool.tile([128, 4], F32, name="bq", tag="bq")
    dma(out=bq_sb[:], in_=d["bq"].ap())
    bk_sb = cpool.tile([128, 4], F32, name="bk", tag="bk")
    dma(out=bk_sb[:], in_=d["bk"].ap())
    bv_sb = cpool.tile([128, 520], F32, name="bv", tag="bv")
    dma(out=bv_sb[:], in_=d["bv"].ap())
    dma(out=wp_sb[:], in_=d["wproj"].ap().rearrange("(pp p) n -> p pp n", p=128))

    qT = [persist.tile([128, T], BF16, name=f"qT{p}", tag=f"qT{p}") for p in range(4)]
    kT = [persist.tile([128, T], BF16, name=f"kT{p}", tag=f"kT{p}") for p in range(4)]
    Vt = [persist.tile([128, 520], BF16, name=f"V{i}", tag=f"V{i}") for i in range(TT)]
    yT = [persist.tile([128, T], BF16, name=f"yT{p}", tag=f"yT{p}") for p in range(4)]

    # ---- emission units ----
    def load_x_chunk(tci):
        def u():
            xt = xts[tci % 2] if tci < 2 else xpool.tile([128, CT, 512], BF16, name="xt", tag="xt")
            if tci >= 2:
                dma(out=xt[:], in_=d["xT"].ap()[:, 512 * tci : 512 * tci + 512].rearrange("(ct p) n -> p ct n", p=128))
            xts.append(xt) if tci >= 2 else None
            _xcur[tci] = xt
        return u

    _xcur: dict = {0: xts[0], 1: xts[1]}

    def qkv_units(tci):
        units = []
        if tci == 1:
            def u_load1():
                dma(out=xts[1][:], in_=d["xT"].ap()[:, 512:1024].rearrange("(ct p) n -> p ct n", p=128))
            units.append(u_load1)
        elif tci >= 2:
            units.append(load_x_chunk(tci))

        for iw, w_sb in ((0, wq_sb), (1, wk_sb)):
            for p in range(4):
                def u(p=p, iw=iw, w_sb=w_sb, tci=tci):
                    xt = _xcur[tci]
                    ps = pgen.tile([128, 512], F32, name="pg", tag="pg")
                    for ct in range(CT):
                        nc.tensor.matmul(
                            ps[:],
                            w_sb[:, ct, 128 * p : 128 * p + 128],
                            xt[:, ct, :],
                            start=(ct == 0),
                            stop=(ct == CT - 1),
                        )
                    if iw == 0:
                        nc.vector.tensor_scalar(
                            qT[p][:, 512 * tci : 512 * tci + 512],
                            ps[:], 0.125, bq_sb[:, p : p + 1], mult, add,
                        )
                    else:
                        nc.vector.tensor_scalar(
                            kT[p][:, 512 * tci : 512 * tci + 512],
                            ps[:], 1.0, bk_sb[:, p : p + 1], mult, add,
                        )
                units.append(u)
        for tt in range(4):
            for qd in range(2):
                def u(tt=tt, qd=qd, tci=tci):
                    xt = _xcur[tci]
                    ps = pgen.tile([128, 512], F32, name="pg", tag="pg")
                    for ct in range(CT):
                        nc.tensor.matmul(
                            ps[:, 0:260],
                            xt[:, ct, 128 * tt : 128 * tt + 128],
                            wv_sb[:, ct, 260 * qd : 260 * qd + 260],
                            start=(ct == 0),
                            stop=(ct == CT - 1),
                        )
                    nc.vector.tensor_tensor(
                        Vt[4 * tci + tt][:, 260 * qd : 260 * qd + 260],
                        ps[:, 0:260],
                        bv_sb[:, 260 * qd : 260 * qd + 260],
                        add,
                    )
                units.append(u)
        return units

    def attn_units(qc):
        units = []
        yqs_box: list = [None]
        es_box: dict = {}

        def u_alloc(qc=qc):
            yqs_box[0] = [
                psy.tile([65, 512], F32, name=f"yq{h2}", tag=f"yq{h2}") for h2 in (0, 1)
            ]

        for p in range(4):
            nki = 4 * qc + 4
            pend = None
            for ki in range(nki):
                def u_se(p=p, ki=ki, qc=qc, first=(ki == 0)):
                    if first:
                        u_alloc()
                    r = ki - 4 * qc
                    c0 = max(0, 128 * r)
                    sblk = pss.tile([128, 1024], F32, name="sblk", tag="sblk")
                    for h2 in (0, 1):
                        pr = 64 * h2
                        nc.tensor.matmul(
                            sblk[:, 512 * h2 + c0 : 512 * h2 + 512],
                            kT[p][pr : pr + 64, 128 * ki : 128 * ki + 128],
                            qT[p][pr : pr + 64, 512 * qc + c0 : 512 * qc + 512],
                            start=True,
                            stop=True,
                        )
                    es = espool.tile([128, 1024], BF16, name="es", tag="es")
                    if r >= 1:
                        w = 512 - c0
                        sv = sblk[:, c0 : c0 + w]
                        s2 = bass.AP(tensor=sv.tensor, offset=sv.offset,
                                     ap=[list(sv.ap[0]), [512, 2], list(sv.ap[1])])
                        ev = es[:, c0 : c0 + w]
                        e2 = bass.AP(tensor=ev.tensor, offset=ev.offset,
                                     ap=[list(ev.ap[0]), [512, 2], list(ev.ap[1])])
                        nc.scalar.activation(e2, s2, Exp)
                    else:
                        nc.scalar.activation(es[:], sblk[:], Exp)
                    if r >= 0:
                        for h2 in (0, 1):
                            cm = 512 * h2 + c0
                            nc.vector.tensor_tensor(
                                es[:, cm : cm + 128], es[:, cm : cm + 128], m01_sb[:], mult,
                            )
                    es_box[(p, ki)] = es

                def u_pv(p=p, ki=ki, qc=qc, nki=nki):
                    yqs = yqs_box[0]
                    es = es_box.pop((p, ki))
                    c0 = max(0, 128 * (ki - 4 * qc))
                    for h2 in (0, 1):
                        hl = 2 * p + h2
                        nc.tensor.matmul(
                            yqs[h2][:, c0:512],
                            Vt[ki][:, 65 * hl : 65 * hl + 65],
                            es[:, 512 * h2 + c0 : 512 * h2 + 512],
                            start=(ki == 0),
                            stop=(ki == nki - 1),
                            skip_group_check=True,
                        )
                units.append(u_se)
                if pend is not None:
                    units.append(pend)
                pend = u_pv
            units.append(pend)

            def u_flush(p=p, qc=qc):
                yqs = yqs_box[0]
                cs = 512 * qc
                for h2 in (0, 1):
                    pr = 64 * h2
                    rs = rpool.tile([1, 512], F32, name="rs", tag="rs")
                    nc.vector.tensor_copy(out=rs[:], in_=yqs[h2][64:65, :])
                    rr = rpool.tile([1, 512], F32, name="rr", tag="rr")
                    nc.vector.reciprocal_approx_fast(out=rr[:], in_=rs[:])
                    rcb = rpool.tile([64, 512], F32, name="rcb", tag="rcb")
                    nc.gpsimd.partition_broadcast(rcb[:], rr[:], channels=64)
                    nc.vector.tensor_tensor(
                        yT[p][pr : pr + 64, cs : cs + 512],
                        yqs[h2][0:64, :],
                        rcb[:],
                        mult,
                    )
            units.append(u_flush)
        return units

    def proj_units(qc):
        units = []
        for tt in range(4 * qc, 4 * qc + 4):
            for cc in range(2):
                def u(tt=tt, cc=cc):
                    ps = pgen.tile([128, 512], F32, name="pg", tag="pg")
                    for pp in range(4):
                        nc.tensor.matmul(
                            ps[:],
                            yT[pp][:, 128 * tt : 128 * tt + 128],
                            wp_sb[:, pp, 512 * cc : 512 * cc + 512],
                            start=(pp == 0),
                            stop=(pp == 3),
                        )
                    ob = opool.tile([128, 512], F32, name="ob", tag="ob")
                    nc.vector.tensor_copy(out=ob[:], in_=ps[:])
                    dma(out=d["out"].ap()[128 * tt : 128 * tt + 128, 512 * cc : 512 * cc + 512], in_=ob[:])
                units.append(u)
        return units

    def weave(primary, secondary):
        if not secondary:
            return list(primary)
        out = []
        n, m = len(primary), len(secondary)
        j = 0
        for i, u in enumerate(primary):
            out.append(u)
            want = (i + 1) * m // n
            while j < want:
                out.append(secondary[j])
                j += 1
        out.extend(secondary[j:])
        return out

    for u in qkv_units(0):
        u()
    for qc in range(TC):
        sec = qkv_units(qc + 1) if qc + 1 < TC else []
        if qc >= 1:
            sec = sec + proj_units(qc - 1)
        for u in weave(attn_units(qc), sec):
            u()
    for u in proj_units(TC - 1):
        u()

    for pool in (pgen, psy, pss, opool, rpool, espool, xpool, wpool, persist, cpool):
        pool.release()


def _build():
    nc = bacc.Bacc("TRN2", target_bir_lowering=False, debug=False, num_devices=N_CORES)
    d = {
        "xT": nc.dram_tensor("xT", [C, T], BF16, kind="ExternalInput"),
        "wq": nc.dram_tensor("wq", [C, 512], BF16, kind="ExternalInput"),
        "wk": nc.dram_tensor("wk", [C, 512], BF16, kind="ExternalInput"),
        "wv": nc.dram_tensor("wv", [C, 520], BF16, kind="ExternalInput"),
        "bv": nc.dram_tensor("bv", [128, 520], F32, kind="ExternalInput"),
        "bq": nc.dram_tensor("bq", [128, 4], F32, kind="ExternalInput"),
        "bk": nc.dram_tensor("bk", [128, 4], F32, kind="ExternalInput"),
        "m01": nc.dram_tensor("m01", [128, 128], BF16, kind="ExternalInput"),
        "wproj": nc.dram_tensor("wproj", [512, 1024], BF16, kind="ExternalInput"),
        "out": nc.dram_tensor("out", [T, C], F32, kind="ExternalOutput"),
    }
    with tile.TileContext(nc) as tcx:
        _emit(nc, tcx, d)
    nc.compile()
    return nc


def _prep_core_inputs(c, x, w_attn, b_attn):
    import ml_dtypes

    bf = ml_dtypes.bfloat16
    g = c % 2
    xT = np.ascontiguousarray(x[c // 2].T).astype(bf)
    wq = np.ascontiguousarray(w_attn[:, 512 * g : 512 * g + 512]).astype(bf)
    wk = np.ascontiguousarray(w_attn[:, 1024 + 512 * g : 1024 + 512 * g + 512]).astype(bf)
    wv = np.zeros((C, 520), np.float32)
    bv = np.zeros((128, 520), np.float32)
    for hl in range(8):
        hcol = 2048 + 512 * g + 64 * hl
        wv[:, 65 * hl : 65 * hl + 64] = w_attn[:, hcol : hcol + 64]
        bv[:, 65 * hl : 65 * hl + 64] = b_attn[hcol : hcol + 64][None, :]
        bv[:, 65 * hl + 64] = 1.0
    bq = np.zeros((128, 4), np.float32)
    bk = np.zeros((128, 4), np.float32)
    for p in range(4):
        bq[:, p] = b_attn[512 * g + 128 * p : 512 * g + 128 * p + 128] * 0.125
        bk[:, p] = b_attn[1024 + 512 * g + 128 * p : 1024 + 512 * g + 128 * p + 128]
    m01 = (np.arange(128)[:, None] <= np.arange(128)[None, :]).astype(bf)
    return dict(xT=xT, wq=wq, wk=wk, wv=wv.astype(bf), bv=bv, bq=bq, bk=bk, m01=m01)


def make_in_maps(x, w_attn, b_attn, w_proj):
    import ml_dtypes

    x = np.asarray(x, np.float32)
    w_attn = np.asarray(w_attn, np.float32)
    b_attn = np.asarray(b_attn, np.float32)
    w_proj = np.asarray(w_proj, np.float32)
    in_maps = []
    for c in range(N_CORES):
        m = _prep_core_inputs(c, x, w_attn, b_attn)
        g = c % 2
        m["wproj"] = np.ascontiguousarray(w_proj[512 * g : 512 * g + 512, :]).astype(
            ml_dtypes.bfloat16
        )
        in_maps.append(m)
    return in_maps


def get_nc():
    if "nc" not in _cache:
        _cache["nc"] = _build()
    return _cache["nc"]


def gather(results, b_proj):
    b_proj = np.asarray(b_proj, np.float32)
    full = np.empty((B, T, C), np.float32)
    for b in range(B):
        full[b] = results[2 * b]["out"] + results[2 * b + 1]["out"] + b_proj[None, :]
    return full


def kernel(x, w_attn, b_attn, w_proj, b_proj):
    nc = get_nc()
    in_maps = make_in_maps(x, w_attn, b_attn, w_proj)
    res = run_bass_kernel_spmd(nc, in_maps, list(range(N_CORES)))
    return gather(res.results, b_proj)
